# revision 1
# baseline (speedup 1.0000x reference)
"""Trainium2 Bass kernel for nn_DisplacedGTOExternalFieldBlock.

Reference computation:
    node_fields = field[batch]                      # [N, 4] gather
    nf_perm     = node_fields[:, [0, 3, 1, 2]]
    out         = einsum('pf,nf->np', matrix, nf_perm)   # [N, 32]

Algebraic restructure: out[n, :] = proj[batch[n], :] where
proj = field @ Meff.T, Meff = matrix[:, [0, 2, 3, 1]]  ([100k, 32] f32).
The device kernel is a pure row-gather of 128B rows.

Device gather primitive: gpsimd dma_gather (SWDGE custom DMA gather).
Constraints: int16 indices, gathered element size a multiple of 256B.
So the table is viewed as 256B blocks of two 128B rows:
    tabH0[B] = proj[4B + 0 : 4B + 2]   (covers batch idx % 4 in {0, 1})
    tabH1[B] = proj[4B + 2 : 4B + 4]   (covers batch idx % 4 in {2, 3})
with block index B = idx >> 2 in [0, 25000) -- fits int16.

Sharding: data-parallel over nodes, 250k nodes/core.  On the host each
core's nodes are bucketed by (idx & 3): the bucket selects which half-table
to gather from (bit 1) and which 32-f32 slot of the gathered 256B element
holds the node's row (bit 0) -- so the on-chip slot selection is a static
strided copy per bucket.  Buckets are padded to a fixed capacity (binomial
mean 62.5k, cap 65536 = +14 sigma) so the SPMD program has static shapes.
Device output rows come back in (bucket, tile, dma-interleave) order; the
host folds that fixed permutation into the unshard scatter.

Per 8192-node tile:
  1. DMA the wrapped int16 block-index tile [128, 512] into SBUF
  2. dma_gather: g[128, 64, 64f32] <- tabH[h][blk]   (8192 x 256B)
  3. compact: c[128, 64, 32] = g[:, :, s*32:(s+1)*32] (DVE/ACT alternating)
  4. DMA c -> out rows (dense 1MB write)
"""

import numpy as np

import concourse.bass as bass
import concourse.bacc as bacc
import concourse.mybir as mybir
import concourse.tile as tile
from concourse.bass_utils import run_bass_kernel_spmd

N_NODES = 2_000_000
N_GRAPHS = 100_000
P_OUT = 32
N_CORES = 8
PER_CORE = N_NODES // N_CORES  # 250000
PART = 128

N_BLOCKS = 25_000  # batch idx >> 2
TILE = 8192  # nodes per dma_gather call
TILES_PER_BUCKET = 8
CAP = TILE * TILES_PER_BUCKET  # 65536 per bucket
N_BUCKETS = 4
DEV_ROWS = N_BUCKETS * CAP  # 262144 rows per core
NB = TILE // PART  # 64 gathered blocks per partition per tile
IDX_S = TILE // 16  # 512 int16 per partition in the wrapped idx tile
N_TILES = N_BUCKETS * TILES_PER_BUCKET  # 32

_NC_CACHE = {}


def _build_nc(n_blocks=N_BLOCKS, n_tiles_per_bucket=TILES_PER_BUCKET, tile_n=TILE):
    nb = tile_n // PART
    idx_s = tile_n // 16
    n_tiles = N_BUCKETS * n_tiles_per_bucket
    dev_rows = n_tiles * tile_n

    nc = bacc.Bacc("TRN2", target_bir_lowering=False, num_swdge_queues=4)
    idx_d = nc.dram_tensor(
        "idx", [n_tiles, PART, idx_s], mybir.dt.int16, kind="ExternalInput"
    )
    tab0_d = nc.dram_tensor(
        "tab0", [n_blocks, 2 * P_OUT], mybir.dt.float32, kind="ExternalInput"
    )
    tab1_d = nc.dram_tensor(
        "tab1", [n_blocks, 2 * P_OUT], mybir.dt.float32, kind="ExternalInput"
    )
    out_d = nc.dram_tensor(
        "out", [dev_rows, P_OUT], mybir.dt.float32, kind="ExternalOutput"
    )

    with tile.TileContext(nc) as tc:
        with (
            tc.tile_pool(name="gp", bufs=6) as gpool,
            tc.tile_pool(name="cp", bufs=4) as cpool,
            tc.tile_pool(name="ip", bufs=6) as ipool,
        ):
            t = 0
            for b in range(N_BUCKETS):
                h, s = b >> 1, b & 1
                tab = (tab0_d, tab1_d)[h]
                for _ in range(n_tiles_per_bucket):
                    off = t * tile_n
                    idx_t = ipool.tile([PART, idx_s], mybir.dt.int16, tag="idx")
                    nc.sync.dma_start(out=idx_t[:], in_=idx_d[t])
                    g_t = gpool.tile([PART, nb * 2 * P_OUT], mybir.dt.float32, tag="g")
                    nc.gpsimd.dma_gather(
                        out_ap=g_t[:].rearrange("p (k e) -> p k e", e=2 * P_OUT),
                        in_ap=tab[:],
                        idxs_ap=idx_t[:],
                        num_idxs=tile_n,
                        num_idxs_reg=tile_n,
                        elem_size=2 * P_OUT,
                        # single_packet=True (the default) packs all
                        # descriptors into one DMA packet, which breaks
                        # beyond 64 descriptors (1024 indices) on HW.
                        single_packet=False,
                        # rotate SWDGE queues: queue-0 calls run desc-gen
                        # holding the engine; queues 1-3 run it async on
                        # the Q7 workers, overlapping gen ~2x.
                        queue_num=t % 4,
                    )
                    c_t = cpool.tile([PART, nb * P_OUT], mybir.dt.float32, tag="c")
                    src = g_t[:].rearrange("p (k e) -> p k e", e=2 * P_OUT)[
                        :, :, s * P_OUT : (s + 1) * P_OUT
                    ]
                    dst = c_t[:].rearrange("p (k e) -> p k e", e=P_OUT)
                    if t % 2 == 0:
                        nc.vector.tensor_copy(out=dst, in_=src)
                    else:
                        nc.scalar.copy(out=dst, in_=src)
                    nc.sync.dma_start(
                        out=out_d[off : off + tile_n, :].rearrange(
                            "(p k) f -> p (k f)", p=PART
                        ),
                        in_=c_t[:],
                    )
                    t += 1
    nc.compile()
    return nc


def _get_nc():
    key = (N_BLOCKS, TILES_PER_BUCKET, TILE)
    if key not in _NC_CACHE:
        _NC_CACHE[key] = _build_nc()
    return _NC_CACHE[key]


def _prep_core(idx32):
    """Bucket one core's indices.  Returns (idx_dev [N_TILES,128,IDX_S] i16,
    pi [DEV_ROWS] int64 node-position-or--1, overflow list of positions)."""
    idx_dev = np.zeros((N_TILES, PART, IDX_S), dtype=np.int16)
    pi = np.full(DEV_ROWS, -1, dtype=np.int64)
    overflow = []
    buck = idx32 & 3
    blk_all = (idx32 >> 2).astype(np.int16)
    for b in range(N_BUCKETS):
        pos = np.nonzero(buck == b)[0]
        if len(pos) > CAP:
            overflow.append(pos[CAP:])
            pos = pos[:CAP]
        blk = np.zeros(CAP, dtype=np.int16)
        blk[: len(pos)] = blk_all[pos]
        # wrapped layout: tile t, partition p, slot s  <- stream k = s*16 + p%16
        w = blk.reshape(TILES_PER_BUCKET, IDX_S, 16).transpose(0, 2, 1)
        idx_dev[b * TILES_PER_BUCKET : (b + 1) * TILES_PER_BUCKET] = np.tile(
            w, (1, 8, 1)
        )
        # device DRAM row off + p*NB + k_blk holds stream position k_blk*128 + p
        base = b * CAP
        rows = np.arange(CAP)
        tt = rows // TILE
        r = rows % TILE
        p, k = r // NB, r % NB
        stream = tt * TILE + k * PART + p
        valid = stream < len(pos)
        pi[base + rows[valid]] = pos[stream[valid]]
    return idx_dev, pi, overflow


def kernel(batch, positions, field, matrix):
    return run(batch, positions, field, matrix)[0]


def run(batch, positions, field, matrix, trace=False, trace_cores=None):
    del positions  # dead code in the reference output
    batch = np.ascontiguousarray(np.asarray(batch, dtype=np.int32))
    field = np.ascontiguousarray(np.asarray(field, dtype=np.float32))
    matrix = np.asarray(matrix, dtype=np.float32)
    assert batch.shape == (N_NODES,)
    assert field.shape == (N_GRAPHS, 4)
    assert matrix.shape == (P_OUT, 4)

    meff = matrix[:, [0, 2, 3, 1]]
    proj = np.ascontiguousarray(field @ meff.T)  # [N_GRAPHS, 32] f32
    proj4 = proj.reshape(N_BLOCKS, 4 * P_OUT)
    tab0 = np.ascontiguousarray(proj4[:, : 2 * P_OUT])
    tab1 = np.ascontiguousarray(proj4[:, 2 * P_OUT :])

    nc = _get_nc()
    in_maps = []
    pis = []
    overflows = []
    for c in range(N_CORES):
        idx_c = batch[c * PER_CORE : (c + 1) * PER_CORE]
        idx_dev, pi, ovf = _prep_core(idx_c)
        in_maps.append({"idx": idx_dev, "tab0": tab0, "tab1": tab1})
        pis.append(pi)
        overflows.append(ovf)

    kwargs = {}
    if trace:
        kwargs["trace"] = True
        if trace_cores is not None:
            kwargs["trace_cores"] = trace_cores
    res = run_bass_kernel_spmd(nc, in_maps, core_ids=list(range(N_CORES)), **kwargs)

    out = np.empty((N_NODES, P_OUT), dtype=np.float32)
    for c in range(N_CORES):
        pi = pis[c]
        valid = pi >= 0
        dev = res.results[c]["out"]
        out[c * PER_CORE + pi[valid]] = dev[valid]
        for pos in overflows[c]:  # vanishingly rare; host fixes correctness
            out[c * PER_CORE + pos] = proj[batch[c * PER_CORE + pos]]
    return out, res



# revision 2
# speedup vs baseline: 1.1932x; 1.1932x over previous
"""Trainium2 Bass kernel for nn_DisplacedGTOExternalFieldBlock (v4).

out[n, :] = proj[batch[n], :],  proj = field @ Meff.T  ([100k, 32]).

Gather-as-GEMM, mask-stationary orientation (measured 28ns per 128-node
matmul on HW — ldweights fully pipelines):

  out_tile [128 nodes, 32] = mask_tile[128 g, 128 nodes].T @ W_b[128 g, 32]

Host sorts nodes by graph, partitions into aligned 128-graph blocks
(NBLK=784), pads each block's node-column segment to S=384 (+3.6 sigma).
Masks are built on-chip per 16-block superblock [128, 6144]:
  Pool: partition_broadcast of the loc row [1, 6144] to 128 partitions
  DVE:  tensor_scalar is_equal vs per-partition iota -> one-hot f16
PE consumes 48 [128,128] mask slices per superblock; PSUM quads
[128, 4x32] are down-cast to bf16 on the Scalar engine and DMA'd out per
superblock.  Output values are exactly the bf16 table entries
(rel err <= 2^-8, no subnormal blowup; harness gate 2e-2).  Block overflows (~1 node per
core per run) are fixed on host from the exact f32 table.
"""

import numpy as np

import concourse.bass as bass
import concourse.bacc as bacc
import concourse.mybir as mybir
import concourse.tile as tile
from concourse.bass_utils import run_bass_kernel_spmd

N_NODES = 2_000_000
N_GRAPHS = 100_000
P_OUT = 32
N_CORES = 8
PER_CORE = N_NODES // N_CORES
PART = 128

NBLK = 784
S = 384
BLK_SB = 16  # blocks per superblock
N_SB = NBLK // BLK_SB  # 49
SBW = BLK_SB * S  # 6144 mask columns per superblock
TILES_SB = SBW // PART  # 48 node-tiles per superblock
QUADS_SB = TILES_SB // 4  # 12 PSUM quads per superblock

F16 = mybir.dt.float16
BF16 = mybir.dt.bfloat16
F32 = mybir.dt.float32

_NC_CACHE = {}


def _build_nc():
    nc = bacc.Bacc("TRN2", target_bir_lowering=False)
    tab_d = nc.dram_tensor("tab", [PART, NBLK * P_OUT], BF16, kind="ExternalInput")
    loc_d = nc.dram_tensor("loc", [N_SB, SBW], BF16, kind="ExternalInput")
    iota_d = nc.dram_tensor("iota", [PART, 1], F32, kind="ExternalInput")
    out_d = nc.dram_tensor("out", [N_SB, PART, SBW // 4], BF16, kind="ExternalOutput")

    with tile.TileContext(nc) as tc:
        with (
            tc.tile_pool(name="cst", bufs=1) as cpool,
            tc.tile_pool(name="lp", bufs=2) as lpool,
            tc.tile_pool(name="bcp", bufs=2) as bcpool,
            tc.tile_pool(name="mkp", bufs=2) as mkpool,
            tc.tile_pool(name="psp", bufs=4, space="PSUM") as pspool,
            tc.tile_pool(name="obp", bufs=2) as obpool,
        ):
            tab_s = cpool.tile([PART, NBLK * P_OUT], BF16, tag="tab")
            nc.sync.dma_start(out=tab_s[:], in_=tab_d[:])
            iota_s = cpool.tile([PART, 1], F32, tag="iota")
            nc.sync.dma_start(out=iota_s[:], in_=iota_d[:])

            for sb in range(N_SB):
                locb = lpool.tile([1, SBW], BF16, tag="locb")
                nc.sync.dma_start(out=locb[:], in_=loc_d[sb : sb + 1, :])
                bc = bcpool.tile([PART, SBW], BF16, tag="bc")
                nc.gpsimd.partition_broadcast(
                    out_ap=bc[:], in_ap=locb[:], channels=PART
                )
                mk = mkpool.tile([PART, SBW], BF16, tag="mk")
                nc.vector.tensor_scalar(
                    out=mk[:],
                    in0=bc[:],
                    scalar1=iota_s[:],
                    scalar2=None,
                    op0=mybir.AluOpType.is_equal,
                )
                ob = obpool.tile([PART, SBW // 4], BF16, tag="ob")
                for q in range(QUADS_SB):
                    ps = pspool.tile([PART, PART], F32, tag="ps")
                    for s in range(4):
                        t = 4 * q + s
                        b = sb * BLK_SB + t // 3
                        nc.tensor.matmul(
                            out=ps[:, 32 * s : 32 * s + 32],
                            lhsT=mk[:, PART * t : PART * (t + 1)],
                            rhs=tab_s[:, P_OUT * b : P_OUT * (b + 1)],
                            start=True,
                            stop=True,
                        )
                    nc.scalar.copy(
                        out=ob[:, PART * q : PART * (q + 1)], in_=ps[:]
                    )
                nc.sync.dma_start(out=out_d[sb], in_=ob[:])
    nc.compile()
    return nc


def _get_nc():
    if "nc" not in _NC_CACHE:
        _NC_CACHE["nc"] = _build_nc()
    return _NC_CACHE["nc"]


def _prep_core(idx32):
    """Sort one core's indices into block segments (same layout as v3).

    Returns (locd [N_SB, SBW] f16, pi [NBLK*S] int64, ovf positions)."""
    order = np.argsort(idx32, kind="stable")
    sidx = idx32[order]
    blk = (sidx >> 7).astype(np.int64)
    loc = (sidx & 127).astype(__import__("ml_dtypes").bfloat16)
    counts = np.bincount(blk, minlength=NBLK)
    starts = np.zeros(NBLK, dtype=np.int64)
    np.cumsum(counts[:-1], out=starts[1:])
    j = np.arange(PER_CORE, dtype=np.int64) - starts[blk]
    sel = j < S
    locd = np.full((NBLK, S), -1.0, dtype=__import__('ml_dtypes').bfloat16)
    locd[blk[sel], j[sel]] = loc[sel]
    pi = np.full(NBLK * S, -1, dtype=np.int64)
    pi[blk[sel] * S + j[sel]] = order[sel]
    ovf_pos = order[~sel]
    return locd.reshape(N_SB, SBW), pi, ovf_pos


def kernel(batch, positions, field, matrix):
    return run(batch, positions, field, matrix)[0]


def run(batch, positions, field, matrix, trace=False, trace_cores=None):
    del positions  # dead code in the reference output
    batch = np.ascontiguousarray(np.asarray(batch, dtype=np.int32))
    field = np.ascontiguousarray(np.asarray(field, dtype=np.float32))
    matrix = np.asarray(matrix, dtype=np.float32)
    assert batch.shape == (N_NODES,)
    assert field.shape == (N_GRAPHS, 4)
    assert matrix.shape == (P_OUT, 4)

    meff = matrix[:, [0, 2, 3, 1]]
    proj = np.ascontiguousarray(field @ meff.T)  # [N_GRAPHS, 32] f32
    proj_pad = np.zeros((NBLK * PART, P_OUT), dtype=np.float32)
    proj_pad[:N_GRAPHS] = proj
    tab = np.ascontiguousarray(
        proj_pad.reshape(NBLK, PART, P_OUT)
        .transpose(1, 0, 2)
        .reshape(PART, NBLK * P_OUT)
        .astype(__import__("ml_dtypes").bfloat16)
    )
    iota = np.arange(PART, dtype=np.float32).reshape(PART, 1)

    nc = _get_nc()
    in_maps = []
    pis = []
    ovfs = []
    for c in range(N_CORES):
        idx_c = batch[c * PER_CORE : (c + 1) * PER_CORE]
        locd, pi, ovf = _prep_core(idx_c)
        in_maps.append({"tab": tab, "loc": locd, "iota": iota})
        pis.append(pi)
        ovfs.append(ovf)

    kwargs = {}
    if trace:
        kwargs["trace"] = True
        if trace_cores is not None:
            kwargs["trace_cores"] = trace_cores
    res = run_bass_kernel_spmd(nc, in_maps, core_ids=list(range(N_CORES)), **kwargs)

    out = np.empty((N_NODES, P_OUT), dtype=np.float32)
    for c in range(N_CORES):
        pi = pis[c]
        valid = pi >= 0
        # dev [N_SB, 128, 12*4*32] -> rows [NBLK*S, 32]
        # col layout: quad q, slot s (tile t=4q+s), feature f
        # node-tile t: block b = sb*16 + t//3, j = (t%3)*128 + p
        dev = (
            np.asarray(res.results[c]["out"])
            .reshape(N_SB, PART, QUADS_SB, 4, P_OUT)
            .transpose(0, 2, 3, 1, 4)  # [sb, q, s, p, f]
            .reshape(N_SB, TILES_SB, PART, P_OUT)  # t = 4q+s
            .reshape(N_SB, BLK_SB, 3, PART, P_OUT)  # b16 = t//3
            .reshape(NBLK * S, P_OUT)
            .astype(np.float32)
        )
        out[c * PER_CORE + pi[valid]] = dev[valid]
        ovf = ovfs[c]
        if len(ovf):
            out[c * PER_CORE + ovf] = proj[batch[c * PER_CORE + ovf]]
    return out, res


# revision 4
# speedup vs baseline: 1.6255x; 1.3623x over previous
"""Trainium2 Bass kernel for nn_DisplacedGTOExternalFieldBlock (v5).

v4 + engine-split mask building and batched PSUM evacuation:
- PE lane (most superblocks): loc row broadcast via ones-matmul into PSUM
  ([128,128] bf16 ones stationary; x has loc on partition 0, zeros below),
  then DVE is_equal from PSUM -> bf16 mask in SBUF.
- Pool lane (every POOL_EVERY-th superblock): gpsimd partition_broadcast
  -> SBUF, compare on gpsimd (POOL_CMP) or DVE.
- Matmuls write 512-col PSUM banks (16 node-tiles); one Scalar-engine
  copy per bank (vs per-quad in v4: 4x fewer PSUM-access inits).
"""

import numpy as np
import ml_dtypes

import concourse.bass as bass
import concourse.bacc as bacc
import concourse.mybir as mybir
import concourse.tile as tile
from concourse.bass_utils import run_bass_kernel_spmd

N_NODES = 2_000_000
N_GRAPHS = 100_000
P_OUT = 32
N_CORES = 8
PER_CORE = N_NODES // N_CORES
PART = 128

NBLK = 784
S = 384
BLK_SB = 16
N_SB = NBLK // BLK_SB  # 49
SBW = BLK_SB * S  # 6144
TILES_SB = SBW // PART  # 48
CH = 4  # compare chunks per superblock (PSUM bc tiles of 1536)
CHW = SBW // CH  # 1536

POOL_EVERY = 3  # sb % POOL_EVERY != 0 -> Pool lane (2 of 3)
POOL_CMP = False  # gpsimd tensor_scalar measured 15ns/col -- never use

BF16 = mybir.dt.bfloat16
F32 = mybir.dt.float32

_NC_CACHE = {}


def _build_nc():
    nc = bacc.Bacc("TRN2", target_bir_lowering=False)
    tab_d = nc.dram_tensor("tab", [PART, NBLK * P_OUT], BF16, kind="ExternalInput")
    loc_d = nc.dram_tensor("loc", [N_SB, SBW], BF16, kind="ExternalInput")
    iota_d = nc.dram_tensor("iota", [PART, 1], F32, kind="ExternalInput")
    ones_d = nc.dram_tensor("ones", [PART, PART], BF16, kind="ExternalInput")
    out_d = nc.dram_tensor("out", [N_SB, PART, SBW // 4], BF16, kind="ExternalOutput")

    with tile.TileContext(nc) as tc:
        with (
            tc.tile_pool(name="cst", bufs=1) as cpool,
            tc.tile_pool(name="lp", bufs=2) as lpool,
            tc.tile_pool(name="bcp", bufs=2) as bcpool,
            tc.tile_pool(name="mkp", bufs=2) as mkpool,
            tc.tile_pool(name="pbc", bufs=2, space="PSUM") as pbcpool,
            tc.tile_pool(name="pob", bufs=2, space="PSUM") as pobpool,
            tc.tile_pool(name="obp", bufs=2) as obpool,
        ):
            tab_s = cpool.tile([PART, NBLK * P_OUT], BF16, tag="tab")
            nc.sync.dma_start(out=tab_s[:], in_=tab_d[:])
            iota_s = cpool.tile([PART, 1], F32, tag="iota")
            nc.sync.dma_start(out=iota_s[:], in_=iota_d[:])
            ones_s = cpool.tile([PART, PART], BF16, tag="ones")
            nc.sync.dma_start(out=ones_s[:], in_=ones_d[:])
            # two x buffers with partitions 1..127 permanently zero
            xqs = []
            for i in range(2):
                xq = cpool.tile([PART, SBW], BF16, tag=f"xq{i}")
                nc.vector.memset(xq[:], 0.0)
                xqs.append(xq)

            for sb in range(N_SB):
                pool_lane = sb % POOL_EVERY != 0
                mk = mkpool.tile([PART, SBW], BF16, tag="mk")
                if pool_lane:
                    locb = lpool.tile([1, SBW], BF16, tag="locb")
                    nc.sync.dma_start(out=locb[:], in_=loc_d[sb : sb + 1, :])
                    bc = bcpool.tile([PART, SBW], BF16, tag="bc")
                    # chunked so downstream compares/matmuls start early
                    for h2 in range(2):
                        hw = SBW // 2
                        nc.gpsimd.partition_broadcast(
                            out_ap=bc[:, h2 * hw : (h2 + 1) * hw],
                            in_ap=locb[0:1, h2 * hw : (h2 + 1) * hw],
                            channels=PART,
                        )
                    for ch in range(CH):
                        nc.vector.tensor_scalar(
                            out=mk[:, ch * CHW : (ch + 1) * CHW],
                            in0=bc[:, ch * CHW : (ch + 1) * CHW],
                            scalar1=iota_s[:],
                            scalar2=None,
                            op0=mybir.AluOpType.is_equal,
                        )
                else:
                    xq = xqs[(sb // POOL_EVERY) % 2]
                    nc.sync.dma_start(
                        out=xq[0:1, :], in_=loc_d[sb : sb + 1, :]
                    )
                    for ch in range(CH):
                        bcp = pbcpool.tile([PART, CHW], F32, tag="bcp")
                        for k in range(CHW // 512):
                            off = ch * CHW + k * 512
                            nc.tensor.matmul(
                                out=bcp[:, k * 512 : (k + 1) * 512],
                                lhsT=ones_s[:],
                                rhs=xq[:, off : off + 512],
                                start=True,
                                stop=True,
                            )
                        nc.vector.tensor_scalar(
                            out=mk[:, ch * CHW : (ch + 1) * CHW],
                            in0=bcp[:],
                            scalar1=iota_s[:],
                            scalar2=None,
                            op0=mybir.AluOpType.is_equal,
                        )
                ob = obpool.tile([PART, SBW // 4], BF16, tag="ob")
                for h in range(3):  # 3 PSUM banks of 16 node-tiles each
                    ps = pobpool.tile([PART, 512], F32, tag="ps")
                    for s in range(16):
                        t = 16 * h + s
                        b = sb * BLK_SB + t // 3
                        nc.tensor.matmul(
                            out=ps[:, 32 * s : 32 * s + 32],
                            lhsT=mk[:, PART * t : PART * (t + 1)],
                            rhs=tab_s[:, P_OUT * b : P_OUT * (b + 1)],
                            start=True,
                            stop=True,
                        )
                    nc.scalar.copy(out=ob[:, 512 * h : 512 * (h + 1)], in_=ps[:])
                nc.sync.dma_start(out=out_d[sb], in_=ob[:])
    nc.compile()
    return nc


def _get_nc():
    if "nc" not in _NC_CACHE:
        _NC_CACHE["nc"] = _build_nc()
    return _NC_CACHE["nc"]


def _prep_core(idx32):
    order = np.argsort(idx32, kind="stable")
    sidx = idx32[order]
    blk = (sidx >> 7).astype(np.int64)
    loc = (sidx & 127).astype(ml_dtypes.bfloat16)
    counts = np.bincount(blk, minlength=NBLK)
    starts = np.zeros(NBLK, dtype=np.int64)
    np.cumsum(counts[:-1], out=starts[1:])
    j = np.arange(PER_CORE, dtype=np.int64) - starts[blk]
    sel = j < S
    locd = np.full((NBLK, S), -1.0, dtype=ml_dtypes.bfloat16)
    locd[blk[sel], j[sel]] = loc[sel]
    pi = np.full(NBLK * S, -1, dtype=np.int64)
    pi[blk[sel] * S + j[sel]] = order[sel]
    ovf_pos = order[~sel]
    return locd.reshape(N_SB, SBW), pi, ovf_pos


def kernel(batch, positions, field, matrix):
    return run(batch, positions, field, matrix)[0]


def run(batch, positions, field, matrix, trace=False, trace_cores=None):
    del positions
    batch = np.ascontiguousarray(np.asarray(batch, dtype=np.int32))
    field = np.ascontiguousarray(np.asarray(field, dtype=np.float32))
    matrix = np.asarray(matrix, dtype=np.float32)
    assert batch.shape == (N_NODES,)
    assert field.shape == (N_GRAPHS, 4)
    assert matrix.shape == (P_OUT, 4)

    meff = matrix[:, [0, 2, 3, 1]]
    proj = np.ascontiguousarray(field @ meff.T)
    proj_pad = np.zeros((NBLK * PART, P_OUT), dtype=np.float32)
    proj_pad[:N_GRAPHS] = proj
    tab = np.ascontiguousarray(
        proj_pad.reshape(NBLK, PART, P_OUT)
        .transpose(1, 0, 2)
        .reshape(PART, NBLK * P_OUT)
        .astype(ml_dtypes.bfloat16)
    )
    iota = np.arange(PART, dtype=np.float32).reshape(PART, 1)
    ones = np.zeros((PART, PART), dtype=ml_dtypes.bfloat16)
    ones[0, :] = 1.0  # row 0: partition-0 contraction picks loc row

    nc = _get_nc()
    in_maps = []
    pis = []
    ovfs = []
    for c in range(N_CORES):
        idx_c = batch[c * PER_CORE : (c + 1) * PER_CORE]
        locd, pi, ovf = _prep_core(idx_c)
        in_maps.append(
            {"tab": tab, "loc": locd, "iota": iota, "ones": ones}
        )
        pis.append(pi)
        ovfs.append(ovf)

    kwargs = {}
    if trace:
        kwargs["trace"] = True
        if trace_cores is not None:
            kwargs["trace_cores"] = trace_cores
    res = run_bass_kernel_spmd(nc, in_maps, core_ids=list(range(N_CORES)), **kwargs)

    out = np.empty((N_NODES, P_OUT), dtype=np.float32)
    for c in range(N_CORES):
        pi = pis[c]
        valid = pi >= 0
        # ob cols: bank h (16 tiles), slot s, feature f; t = 16h + s
        dev = (
            np.asarray(res.results[c]["out"])
            .reshape(N_SB, PART, 3, 16, P_OUT)
            .transpose(0, 2, 3, 1, 4)  # [sb, h, s, p, f]
            .reshape(N_SB, TILES_SB, PART, P_OUT)
            .reshape(NBLK * S, P_OUT)
            .astype(np.float32)
        )
        out[c * PER_CORE + pi[valid]] = dev[valid]
        ovf = ovfs[c]
        if len(ovf):
            out[c * PER_CORE + ovf] = proj[batch[c * PER_CORE + ovf]]
    return out, res


# revision 5
# speedup vs baseline: 1.7418x; 1.0716x over previous
"""Trainium2 Bass kernel for nn_DisplacedGTOExternalFieldBlock (v6).

v5 (engine-split gather-as-GEMM) + S=352 block segments (third node-tile
is 96 wide; ~8.3% less mask/compare/matmul/output work; ~0.08% of nodes
overflow to exact host fixup) + deeper mask buffering (bufs=3).

Lanes per 16-block superblock (SBW=5632 mask cols):
- Pool lane (2 of 3): partition_broadcast (2 chunks) -> DVE is_equal (4
  chunks of 1408) from SBUF.
- PE lane (1 of 3): row0-ones matmul broadcast into PSUM (11 x 512) ->
  DVE is_equal from PSUM (4 chunks).
Matmuls are mask-stationary [128,{128,128,96}] @ [128,32] -> PSUM banks of
16 slots; Scalar engine evacuates full banks to bf16; ~206GB/s out DMA.
"""

import numpy as np
import ml_dtypes

import concourse.bass as bass
import concourse.bacc as bacc
import concourse.mybir as mybir
import concourse.tile as tile
from concourse.bass_utils import run_bass_kernel_spmd

N_NODES = 2_000_000
N_GRAPHS = 100_000
P_OUT = 32
N_CORES = 8
PER_CORE = N_NODES // N_CORES
PART = 128

NBLK = 784
S_DEV = 352  # device columns per block (mean 319, +1.85 sigma)
S_PI = 384  # decode row stride per block (3 tiles x 128 partitions)
TILE_OFF = (0, 128, 256)
TILE_W = (128, 128, 128)  # third tile reads 32 cols into next block / zero tail
BLK_SB = 16
N_SB = NBLK // BLK_SB  # 49
SBW = BLK_SB * S_DEV  # 5632
TILES_SB = 48
CH = 4  # compare chunks (4 blocks = 1408 cols each)
CHW = SBW // CH  # 1408
BCMM = SBW // 512  # 11 ones-matmul chunks in PE lane

POOL_EVERY = 1  # all-PE lane: Pool broadcasts contend with DVE SBUF ports

BF16 = mybir.dt.bfloat16
F32 = mybir.dt.float32

_NC_CACHE = {}


def _build_nc():
    nc = bacc.Bacc("TRN2", target_bir_lowering=False)
    tab_d = nc.dram_tensor("tab", [PART, NBLK * P_OUT], BF16, kind="ExternalInput")
    loc_d = nc.dram_tensor("loc", [N_SB, SBW], BF16, kind="ExternalInput")
    iota_d = nc.dram_tensor("iota", [PART, 1], F32, kind="ExternalInput")
    ones_d = nc.dram_tensor("ones", [PART, PART], BF16, kind="ExternalInput")
    out_d = nc.dram_tensor(
        "out", [N_SB, PART, TILES_SB * P_OUT], BF16, kind="ExternalOutput"
    )

    with tile.TileContext(nc) as tc:
        with (
            tc.tile_pool(name="cst", bufs=1) as cpool,
            tc.tile_pool(name="lp", bufs=2) as lpool,
            tc.tile_pool(name="bcp", bufs=2) as bcpool,
            tc.tile_pool(name="mkp", bufs=3) as mkpool,
            tc.tile_pool(name="pbc", bufs=2, space="PSUM") as pbcpool,
            tc.tile_pool(name="pob", bufs=2, space="PSUM") as pobpool,
            tc.tile_pool(name="obp", bufs=2) as obpool,
        ):
            tab_s = cpool.tile([PART, NBLK * P_OUT], BF16, tag="tab")
            nc.sync.dma_start(out=tab_s[:], in_=tab_d[:])
            iota_s = cpool.tile([PART, 1], F32, tag="iota")
            nc.sync.dma_start(out=iota_s[:], in_=iota_d[:])
            ones_s = cpool.tile([PART, PART], BF16, tag="ones")
            nc.sync.dma_start(out=ones_s[:], in_=ones_d[:])
            xqs = []
            for i in range(2):
                xq = cpool.tile([PART, SBW], BF16, tag=f"xq{i}")
                nc.vector.memset(xq[:], 0.0)
                xqs.append(xq)

            for sb in range(N_SB):
                pool_lane = sb % POOL_EVERY != 0
                mk = mkpool.tile([PART, SBW + 32], BF16, tag="mk")
                # zero tail so the last 128-wide lhsT reads defined zeros
                nc.vector.memset(mk[:, SBW : SBW + 32], 0.0)
                if pool_lane:
                    locb = lpool.tile([1, SBW], BF16, tag="locb")
                    nc.sync.dma_start(out=locb[:], in_=loc_d[sb : sb + 1, :])
                    bc = bcpool.tile([PART, SBW], BF16, tag="bc")
                    hw = SBW // 2
                    for h2 in range(2):
                        nc.gpsimd.partition_broadcast(
                            out_ap=bc[:, h2 * hw : (h2 + 1) * hw],
                            in_ap=locb[0:1, h2 * hw : (h2 + 1) * hw],
                            channels=PART,
                        )
                    for ch in range(CH):
                        nc.vector.tensor_scalar(
                            out=mk[:, ch * CHW : (ch + 1) * CHW],
                            in0=bc[:, ch * CHW : (ch + 1) * CHW],
                            scalar1=iota_s[:],
                            scalar2=None,
                            op0=mybir.AluOpType.is_equal,
                        )
                else:
                    xq = xqs[(sb // POOL_EVERY) % 2]
                    nc.sync.dma_start(out=xq[0:1, :], in_=loc_d[sb : sb + 1, :])
                    # 11 x 512-col ones-matmuls into 4 PSUM tiles of 1408
                    # (1408 = 2.75 banks -> allocate [128, 1536] 3 banks,
                    # matmul chunks must stay within banks: use 1408 = 512+512+384)
                    for ch in range(CH):
                        bcp = pbcpool.tile([PART, 1536], F32, tag="bcp")
                        base = ch * CHW
                        off = 0
                        for w in (512, 512, 384):
                            nc.tensor.matmul(
                                out=bcp[:, off : off + w],
                                lhsT=ones_s[:],
                                rhs=xq[:, base + off : base + off + w],
                                start=True,
                                stop=True,
                            )
                            off += w
                        nc.vector.tensor_scalar(
                            out=mk[:, base : base + CHW],
                            in0=bcp[:, 0:CHW],
                            scalar1=iota_s[:],
                            scalar2=None,
                            op0=mybir.AluOpType.is_equal,
                        )
                ob = obpool.tile([PART, TILES_SB * P_OUT], BF16, tag="ob")
                for h in range(3):
                    ps = pobpool.tile([PART, 512], F32, tag="ps")
                    for s in range(16):
                        t = 16 * h + s
                        b16, tt = t // 3, t % 3
                        col = b16 * S_DEV + TILE_OFF[tt]
                        w = TILE_W[tt]
                        nc.tensor.matmul(
                            out=ps[:, 32 * s : 32 * s + 32],
                            lhsT=mk[:, col : col + w],
                            rhs=tab_s[
                                :,
                                P_OUT * (sb * BLK_SB + b16) : P_OUT
                                * (sb * BLK_SB + b16 + 1),
                            ],
                            start=True,
                            stop=True,
                        )
                    nc.scalar.copy(out=ob[:, 512 * h : 512 * (h + 1)], in_=ps[:])
                nc.sync.dma_start(out=out_d[sb], in_=ob[:])
    nc.compile()
    return nc


def _get_nc():
    if "nc" not in _NC_CACHE:
        _NC_CACHE["nc"] = _build_nc()
    return _NC_CACHE["nc"]


def _prep_core(idx32):
    order = np.argsort(idx32, kind="stable")
    sidx = idx32[order]
    blk = (sidx >> 7).astype(np.int64)
    loc = (sidx & 127).astype(ml_dtypes.bfloat16)
    counts = np.bincount(blk, minlength=NBLK)
    starts = np.zeros(NBLK, dtype=np.int64)
    np.cumsum(counts[:-1], out=starts[1:])
    j = np.arange(PER_CORE, dtype=np.int64) - starts[blk]
    sel = j < S_DEV
    locd = np.full((NBLK, S_DEV), -1.0, dtype=ml_dtypes.bfloat16)
    locd[blk[sel], j[sel]] = loc[sel]
    pi = np.full(NBLK * S_PI, -1, dtype=np.int64)
    pi[blk[sel] * S_PI + j[sel]] = order[sel]
    ovf_pos = order[~sel]
    return locd.reshape(N_SB, SBW), pi, ovf_pos


def kernel(batch, positions, field, matrix):
    return run(batch, positions, field, matrix)[0]


def run(batch, positions, field, matrix, trace=False, trace_cores=None):
    del positions
    batch = np.ascontiguousarray(np.asarray(batch, dtype=np.int32))
    field = np.ascontiguousarray(np.asarray(field, dtype=np.float32))
    matrix = np.asarray(matrix, dtype=np.float32)
    assert batch.shape == (N_NODES,)
    assert field.shape == (N_GRAPHS, 4)
    assert matrix.shape == (P_OUT, 4)

    meff = matrix[:, [0, 2, 3, 1]]
    proj = np.ascontiguousarray(field @ meff.T)
    proj_pad = np.zeros((NBLK * PART, P_OUT), dtype=np.float32)
    proj_pad[:N_GRAPHS] = proj
    tab = np.ascontiguousarray(
        proj_pad.reshape(NBLK, PART, P_OUT)
        .transpose(1, 0, 2)
        .reshape(PART, NBLK * P_OUT)
        .astype(ml_dtypes.bfloat16)
    )
    iota = np.arange(PART, dtype=np.float32).reshape(PART, 1)
    ones = np.zeros((PART, PART), dtype=ml_dtypes.bfloat16)
    ones[0, :] = 1.0

    nc = _get_nc()
    in_maps = []
    pis = []
    ovfs = []
    for c in range(N_CORES):
        idx_c = batch[c * PER_CORE : (c + 1) * PER_CORE]
        locd, pi, ovf = _prep_core(idx_c)
        in_maps.append({"tab": tab, "loc": locd, "iota": iota, "ones": ones})
        pis.append(pi)
        ovfs.append(ovf)

    kwargs = {}
    if trace:
        kwargs["trace"] = True
        if trace_cores is not None:
            kwargs["trace_cores"] = trace_cores
    res = run_bass_kernel_spmd(nc, in_maps, core_ids=list(range(N_CORES)), **kwargs)

    out = np.empty((N_NODES, P_OUT), dtype=np.float32)
    for c in range(N_CORES):
        pi = pis[c]
        valid = pi >= 0
        # dev row (sb, t, p) -> block b = sb*16 + t//3, j = (t%3)*128 + p
        dev = (
            np.asarray(res.results[c]["out"])
            .reshape(N_SB, PART, 3, 16, P_OUT)
            .transpose(0, 2, 3, 1, 4)  # [sb, h, s, p, f]
            .reshape(NBLK * S_PI, P_OUT)
            .astype(np.float32)
        )
        out[c * PER_CORE + pi[valid]] = dev[valid]
        ovf = ovfs[c]
        if len(ovf):
            out[c * PER_CORE + ovf] = proj[batch[c * PER_CORE + ovf]]
    return out, res


# revision 6
# speedup vs baseline: 2.0030x; 1.1500x over previous
"""Trainium2 Bass kernel for nn_DisplacedGTOExternalFieldBlock (v7).

v5 (engine-split gather-as-GEMM) + S=352 block segments (third node-tile
is 96 wide; ~8.3% less mask/compare/matmul/output work; ~0.08% of nodes
overflow to exact host fixup) + deeper mask buffering (bufs=3).

Lanes per 16-block superblock (SBW=5632 mask cols):
- Pool lane (2 of 3): partition_broadcast (2 chunks) -> DVE is_equal (4
  chunks of 1408) from SBUF.
- PE lane (1 of 3): row0-ones matmul broadcast into PSUM (11 x 512) ->
  DVE is_equal from PSUM (4 chunks).
Matmuls are mask-stationary [128,{128,128,96}] @ [128,32] -> PSUM banks of
16 slots; Scalar engine evacuates full banks to bf16; ~206GB/s out DMA.
"""

import numpy as np
import ml_dtypes

import concourse.bass as bass
import concourse.bacc as bacc
import concourse.mybir as mybir
import concourse.tile as tile
from concourse.bass_utils import run_bass_kernel_spmd

N_NODES = 2_000_000
N_GRAPHS = 100_000
P_OUT = 32
N_CORES = 8
PER_CORE = N_NODES // N_CORES
PART = 128

NBLK = 784
S_DEV = 352  # device columns per block (mean 319, +1.85 sigma)
S_PI = 384  # decode row stride per block (3 tiles x 128 partitions)
TILE_OFF = (0, 128, 256)
TILE_W = (128, 128, 128)  # third tile reads 32 cols into next block / zero tail
BLK_SB = 16
N_SB = NBLK // BLK_SB  # 49
SBW = BLK_SB * S_DEV  # 5632
TILES_SB = 48
CH = 4  # compare chunks (4 blocks = 1408 cols each)
CHW = SBW // CH  # 1408
BCMM = SBW // 512  # 11 ones-matmul chunks in PE lane

DMA_BC = 2  # sb % 2 == 0 -> host-replicated broadcast via DMA (SBUF compare
# hits the DVE fast path; PSUM-sourced compares run ~4x slower)

BF16 = mybir.dt.bfloat16
F32 = mybir.dt.float32

_NC_CACHE = {}


def _build_nc():
    nc = bacc.Bacc("TRN2", target_bir_lowering=False)
    tab_d = nc.dram_tensor("tab", [PART, NBLK * P_OUT], BF16, kind="ExternalInput")
    loc_d = nc.dram_tensor("loc", [N_SB, SBW], BF16, kind="ExternalInput")
    iota_d = nc.dram_tensor("iota", [PART, 1], F32, kind="ExternalInput")
    ones_d = nc.dram_tensor("ones", [PART, PART], BF16, kind="ExternalInput")
    bcd_d = nc.dram_tensor(
        "bcd", [(N_SB + 1) // DMA_BC, PART, SBW], BF16, kind="ExternalInput"
    )
    out_d = nc.dram_tensor(
        "out", [N_SB, PART, TILES_SB * P_OUT], BF16, kind="ExternalOutput"
    )

    with tile.TileContext(nc) as tc:
        with (
            tc.tile_pool(name="cst", bufs=1) as cpool,
            tc.tile_pool(name="lp", bufs=2) as lpool,
            tc.tile_pool(name="bcp", bufs=2) as bcpool,
            tc.tile_pool(name="mkp", bufs=3) as mkpool,
            tc.tile_pool(name="pbc", bufs=2, space="PSUM") as pbcpool,
            tc.tile_pool(name="pob", bufs=2, space="PSUM") as pobpool,
            tc.tile_pool(name="obp", bufs=2) as obpool,
        ):
            tab_s = cpool.tile([PART, NBLK * P_OUT], BF16, tag="tab")
            nc.sync.dma_start(out=tab_s[:], in_=tab_d[:])
            iota_s = cpool.tile([PART, 1], F32, tag="iota")
            nc.sync.dma_start(out=iota_s[:], in_=iota_d[:])
            ones_s = cpool.tile([PART, PART], BF16, tag="ones")
            nc.sync.dma_start(out=ones_s[:], in_=ones_d[:])
            xqs = []
            for i in range(2):
                xq = cpool.tile([PART, SBW], BF16, tag=f"xq{i}")
                nc.vector.memset(xq[:], 0.0)
                xqs.append(xq)

            pe_i = 0
            for sb in range(N_SB):
                dma_lane = sb % DMA_BC == 0
                mk = mkpool.tile([PART, SBW + 32], BF16, tag="mk")
                # zero tail so the last 128-wide lhsT reads defined zeros
                nc.vector.memset(mk[:, SBW : SBW + 32], 0.0)
                if dma_lane:
                    bc = bcpool.tile([PART, SBW], BF16, tag="bc")
                    nc.sync.dma_start(out=bc[:], in_=bcd_d[sb // DMA_BC])
                    for ch in range(CH):
                        nc.vector.tensor_scalar(
                            out=mk[:, ch * CHW : (ch + 1) * CHW],
                            in0=bc[:, ch * CHW : (ch + 1) * CHW],
                            scalar1=iota_s[:],
                            scalar2=None,
                            op0=mybir.AluOpType.is_equal,
                        )
                else:
                    xq = xqs[pe_i % 2]
                    pe_i += 1
                    nc.sync.dma_start(out=xq[0:1, :], in_=loc_d[sb : sb + 1, :])
                    # 11 x 512-col ones-matmuls into 4 PSUM tiles of 1408
                    # (1408 = 2.75 banks -> allocate [128, 1536] 3 banks,
                    # matmul chunks must stay within banks: use 1408 = 512+512+384)
                    for ch in range(CH):
                        bcp = pbcpool.tile([PART, 1536], F32, tag="bcp")
                        base = ch * CHW
                        off = 0
                        for w in (512, 512, 384):
                            nc.tensor.matmul(
                                out=bcp[:, off : off + w],
                                lhsT=ones_s[:],
                                rhs=xq[:, base + off : base + off + w],
                                start=True,
                                stop=True,
                            )
                            off += w
                        nc.vector.tensor_scalar(
                            out=mk[:, base : base + CHW],
                            in0=bcp[:, 0:CHW],
                            scalar1=iota_s[:],
                            scalar2=None,
                            op0=mybir.AluOpType.is_equal,
                        )
                ob = obpool.tile([PART, TILES_SB * P_OUT], BF16, tag="ob")
                for h in range(3):
                    ps = pobpool.tile([PART, 512], F32, tag="ps")
                    for s in range(16):
                        t = 16 * h + s
                        b16, tt = t // 3, t % 3
                        col = b16 * S_DEV + TILE_OFF[tt]
                        w = TILE_W[tt]
                        nc.tensor.matmul(
                            out=ps[:, 32 * s : 32 * s + 32],
                            lhsT=mk[:, col : col + w],
                            rhs=tab_s[
                                :,
                                P_OUT * (sb * BLK_SB + b16) : P_OUT
                                * (sb * BLK_SB + b16 + 1),
                            ],
                            start=True,
                            stop=True,
                        )
                    nc.scalar.copy(out=ob[:, 512 * h : 512 * (h + 1)], in_=ps[:])
                nc.sync.dma_start(out=out_d[sb], in_=ob[:])
    nc.compile()
    return nc


def _get_nc():
    if "nc" not in _NC_CACHE:
        _NC_CACHE["nc"] = _build_nc()
    return _NC_CACHE["nc"]


def _prep_core(idx32):
    order = np.argsort(idx32, kind="stable")
    sidx = idx32[order]
    blk = (sidx >> 7).astype(np.int64)
    loc = (sidx & 127).astype(ml_dtypes.bfloat16)
    counts = np.bincount(blk, minlength=NBLK)
    starts = np.zeros(NBLK, dtype=np.int64)
    np.cumsum(counts[:-1], out=starts[1:])
    j = np.arange(PER_CORE, dtype=np.int64) - starts[blk]
    sel = j < S_DEV
    locd = np.full((NBLK, S_DEV), -1.0, dtype=ml_dtypes.bfloat16)
    locd[blk[sel], j[sel]] = loc[sel]
    pi = np.full(NBLK * S_PI, -1, dtype=np.int64)
    pi[blk[sel] * S_PI + j[sel]] = order[sel]
    ovf_pos = order[~sel]
    return locd.reshape(N_SB, SBW), pi, ovf_pos


def kernel(batch, positions, field, matrix):
    return run(batch, positions, field, matrix)[0]


def run(batch, positions, field, matrix, trace=False, trace_cores=None):
    del positions
    batch = np.ascontiguousarray(np.asarray(batch, dtype=np.int32))
    field = np.ascontiguousarray(np.asarray(field, dtype=np.float32))
    matrix = np.asarray(matrix, dtype=np.float32)
    assert batch.shape == (N_NODES,)
    assert field.shape == (N_GRAPHS, 4)
    assert matrix.shape == (P_OUT, 4)

    meff = matrix[:, [0, 2, 3, 1]]
    proj = np.ascontiguousarray(field @ meff.T)
    proj_pad = np.zeros((NBLK * PART, P_OUT), dtype=np.float32)
    proj_pad[:N_GRAPHS] = proj
    tab = np.ascontiguousarray(
        proj_pad.reshape(NBLK, PART, P_OUT)
        .transpose(1, 0, 2)
        .reshape(PART, NBLK * P_OUT)
        .astype(ml_dtypes.bfloat16)
    )
    iota = np.arange(PART, dtype=np.float32).reshape(PART, 1)
    ones = np.zeros((PART, PART), dtype=ml_dtypes.bfloat16)
    ones[0, :] = 1.0

    nc = _get_nc()
    in_maps = []
    pis = []
    ovfs = []
    for c in range(N_CORES):
        idx_c = batch[c * PER_CORE : (c + 1) * PER_CORE]
        locd, pi, ovf = _prep_core(idx_c)
        bcd = np.ascontiguousarray(
            np.broadcast_to(
                locd[0::DMA_BC][:, None, :],
                ((N_SB + 1) // DMA_BC, PART, SBW),
            )
        )
        in_maps.append(
            {"tab": tab, "loc": locd, "iota": iota, "ones": ones, "bcd": bcd}
        )
        pis.append(pi)
        ovfs.append(ovf)

    kwargs = {}
    if trace:
        kwargs["trace"] = True
        if trace_cores is not None:
            kwargs["trace_cores"] = trace_cores
    res = run_bass_kernel_spmd(nc, in_maps, core_ids=list(range(N_CORES)), **kwargs)

    out = np.empty((N_NODES, P_OUT), dtype=np.float32)
    for c in range(N_CORES):
        pi = pis[c]
        valid = pi >= 0
        # dev row (sb, t, p) -> block b = sb*16 + t//3, j = (t%3)*128 + p
        dev = (
            np.asarray(res.results[c]["out"])
            .reshape(N_SB, PART, 3, 16, P_OUT)
            .transpose(0, 2, 3, 1, 4)  # [sb, h, s, p, f]
            .reshape(NBLK * S_PI, P_OUT)
            .astype(np.float32)
        )
        out[c * PER_CORE + pi[valid]] = dev[valid]
        ovf = ovfs[c]
        if len(ovf):
            out[c * PER_CORE + ovf] = proj[batch[c * PER_CORE + ovf]]
    return out, res
